# revision 14
# baseline (speedup 1.0000x reference)
"""Trainium2 Bass kernel for nn_ComplexNetMLP (complex ternary-quant MLP).

Data-parallel over the 8192 rows across 8 NeuronCores (1024 rows each, no
collectives). Strategy vs the earlier version:

- Gate/up complex linears use a 3-matmul Karatsuba decomposition
  (t1 = qxr@qwr.T, t2 = qxi@qwi.T, t3 = (qxr-qxi)@(qwr+qwi).T;
   out_r = t1+t2, out_i = t3-t1+t2). t1/t2 keep exact int8-in-fp16
  operands against exact ternary weights; only the t3 chain uses
  fp16-rounded combined operands (keeps added rel-err ~5e-3, validated
  against the reference in simulation). Cuts gate/up PE work by 25%.
- fp16 on the whole matmul path (same PE rate as bf16, 8x finer mantissa).
- Host-packed per-tile weight layouts: one DMA per weight tile with
  8-22KB contiguous per-partition lines (instead of 16-43 narrow DMAs).
- Epilogue element-wise work spread across DVE + Pool + Act engines
  (Pool was idle before); DMA issue spread across SP/Act/Pool queues.
- The h*ln spill to DRAM is fp16: half the reload traffic on the
  phase-1 -> phase-3 transition, which was a 140us PE bubble.
"""

import sys

sys.path.insert(0, "/opt/trn_rl_repo")

import numpy as np

import concourse.bass as bass
import concourse.tile as tile
from concourse import bacc, mybir
from concourse.bass_utils import run_bass_kernel_spmd

F32 = mybir.dt.float32
F16 = mybir.dt.float16

HIDDEN = 2048
IM = 5504
B, S = 4, 2048
ROWS = B * S              # 8192
NCORES = 8
RPC = ROWS // NCORES      # 1024 rows per core
NRB = RPC // 128          # 8 row-blocks
NHALF = 2
RBH = NRB // NHALF        # 4 row-blocks per half
KO = HIDDEN // 128        # 16 k-chunks for gate/up
IO = IM // 128            # 43 im-chunks for down
EPS = 1e-6
MAGIC = float(np.float32(12582912.0))  # 1.5 * 2**23, RNE rounding magic

P1_W = 256
IM_TILES = [(i * P1_W, min(P1_W, IM - i * P1_W)) for i in range((IM + P1_W - 1) // P1_W)]
NIT = len(IM_TILES)        # 22
HT_W = 256
HID_TILES = [(i * HT_W, HT_W) for i in range(HIDDEN // HT_W)]
NHT = len(HID_TILES)       # 8

Alu = mybir.AluOpType
Act = mybir.ActivationFunctionType


def _weight_prep(wr, wi):
    """Ternary sign matrices + scales (same math as reference.weight_quant)."""
    awr = np.abs(wr)
    awi = np.abs(wi)
    rmask = awr > awi
    imask = ~rmask
    tr = np.where(rmask, np.sign(wr), 0.0).astype(np.float64)
    ti = np.where(imask, np.sign(wi), 0.0).astype(np.float64)
    rcnt = np.maximum(rmask.sum(dtype=np.float64), 1.0)
    icnt = np.maximum(imask.sum(dtype=np.float64), 1.0)
    rmean = np.float32(np.sum(awr * rmask, dtype=np.float64) / rcnt)
    imean = np.float32(np.sum(awi * imask, dtype=np.float64) / icnt)
    rm = float(np.clip(rmean, np.float32(1e-5), None))
    im = float(np.clip(imean, np.float32(1e-5), None))
    return tr, ti, rm, im


def _pack_p1(mat):
    """[IM, HIDDEN] f64 -> per-im-tile [128, KO, w] f16 (ki on partitions)."""
    a = mat.T  # [HIDDEN, IM]
    out = []
    for n0, w in IM_TILES:
        t = a[:, n0:n0 + w].reshape(KO, 128, w).transpose(1, 0, 2)
        out.append(np.ascontiguousarray(t).astype(np.float16))
    return out


def _pack_dn(mat):
    """[HIDDEN, IM] f64 -> per-hid-tile [128, IO, 256] f16 (ii on partitions)."""
    a = mat.T  # [IM, HIDDEN]
    out = []
    for h0, hw in HID_TILES:
        t = a[:, h0:h0 + hw].reshape(IO, 128, hw).transpose(1, 0, 2)
        out.append(np.ascontiguousarray(t).astype(np.float16))
    return out


def _build_program(rm_g, im_g, rm_u, im_u, rm_d, im_d):
    nc = bacc.Bacc("TRN2", target_bir_lowering=False, debug=False,
                   num_devices=NCORES)

    xr_d = nc.dram_tensor("xr", [RPC, HIDDEN], F32, kind="ExternalInput")
    xi_d = nc.dram_tensor("xi", [RPC, HIDDEN], F32, kind="ExternalInput")
    wg = {}
    for key in ("gr", "gi", "gs", "ur", "ui", "us"):
        wg[key] = [nc.dram_tensor(f"w{key}_{it}", [128, KO, w], F16,
                                  kind="ExternalInput")
                   for it, (n0, w) in enumerate(IM_TILES)]
    wd = {}
    for key in ("dr", "di"):
        wd[key] = [nc.dram_tensor(f"w{key}_{ht}", [128, IO, HT_W], F16,
                                  kind="ExternalInput")
                   for ht in range(NHT)]
    lnr_d = nc.dram_tensor("ln_r", [1, IM], F32, kind="ExternalInput")
    lni_d = nc.dram_tensor("ln_i", [1, IM], F32, kind="ExternalInput")
    or_d = nc.dram_tensor("o_r", [RPC, HIDDEN], F32, kind="ExternalOutput")
    oi_d = nc.dram_tensor("o_i", [RPC, HIDDEN], F32, kind="ExternalOutput")

    tr_d = nc.dram_tensor("t_r_scratch", [RPC, IM], F16)
    ti_d = nc.dram_tensor("t_i_scratch", [RPC, IM], F16)

    def ln_bcast(handle, n0, w):
        return bass.AP(tensor=handle, offset=n0, ap=[[0, 128], [1, w]])

    # phase-2 quant io-chunking (43 io chunks of 128 -> 7 chunks)
    QCH = [(0, 6), (6, 6), (12, 6), (18, 6), (24, 6), (30, 6), (36, 7)]

    with tile.TileContext(nc) as tc:
      with tc.tile_pool(name="stats", bufs=1) as stats:
        acc = {k: [[stats.tile([128, 1], F32, name=f"acc{k}{rb}{s}")
                    for s in range(2)] for rb in range(NRB)]
               for k in ("ssr", "ssi", "amr", "ami")}
        last = (NIT - 1) % 2

        for half in range(NHALF):
          rbs = [half * RBH + i for i in range(RBH)]

          # ---------------- prologue + phase 1 ----------------
          with tc.tile_pool(name=f"xqt{half}", bufs=1) as xqt_pool:
            xqt = {"r": {}, "i": {}, "s": {}}
            sg = {}
            mclip = {"r": {}, "i": {}}
            for rb in rbs:
                rr = rb * 128
                with tc.tile_pool(name=f"prol{rb}", bufs=1) as pp:
                    c1t = {}
                    for comp, x_d in (("r", xr_d), ("i", xi_d)):
                        xt = pp.tile([128, HIDDEN], F32, name=f"x{comp}{rb}")
                        nc.scalar.dma_start(xt[:], x_d.ap()[rr:rr + 128, :])
                        am = stats.tile([128, 1], F32, name=f"am{comp}{rb}")
                        nc.vector.tensor_reduce(
                            out=am[:], in_=xt[:], axis=mybir.AxisListType.X,
                            op=Alu.max, apply_absolute_value=True)
                        m = stats.tile([128, 1], F32, name=f"m{comp}{rb}")
                        nc.vector.tensor_scalar_max(out=m[:], in0=am[:],
                                                    scalar1=1e-5)
                        mclip[comp][rb] = m
                        rcp = stats.tile([128, 1], F32, name=f"rcp{comp}{rb}")
                        nc.vector.reciprocal(rcp[:], m[:])
                        c = stats.tile([128, 1], F32, name=f"c{comp}{rb}")
                        nc.vector.tensor_scalar_mul(out=c[:], in0=rcp[:],
                                                    scalar1=127.0)
                        dl = stats.tile([128, 1], F32, name=f"dl{comp}{rb}")
                        nc.vector.tensor_scalar_mul(out=dl[:], in0=m[:],
                                                    scalar1=1.0 / 127.0)
                        s1 = pp.tile([128, HIDDEN], F32, name=f"s1{comp}{rb}")
                        nc.vector.tensor_scalar(
                            out=s1[:], in0=xt[:], scalar1=c[:],
                            scalar2=MAGIC, op0=Alu.mult, op1=Alu.add)
                        xq = pp.tile([128, HIDDEN], F16, name=f"xq{comp}{rb}")
                        nc.scalar.activation(xq[:], s1[:], Act.Copy,
                                             bias=-MAGIC)
                        xqt_t = xqt_pool.tile([128, KO, 128], F16,
                                              name=f"xqt{comp}{rb}")
                        nc.scalar.dma_start_transpose(xqt_t[:], xq[:])
                        xqt[comp][rb] = xqt_t
                        c1 = pp.tile([128, HIDDEN], F32, name=f"c1{comp}{rb}")
                        nc.scalar.activation(c1[:], xq[:], Act.Copy,
                                             scale=dl[:])
                        c1t[comp] = c1
                    b3 = pp.tile([128, HIDDEN], F16, name=f"b3{rb}")
                    nc.vector.tensor_sub(out=b3[:], in0=c1t["r"][:],
                                         in1=c1t["i"][:])
                    b3t = xqt_pool.tile([128, KO, 128], F16, name=f"b3t{rb}")
                    nc.scalar.dma_start_transpose(b3t[:], b3[:])
                    xqt["s"][rb] = b3t
                scl = {}
                for nm, const, m in (
                    ("g1", rm_g, mclip["r"][rb]), ("g2", im_g, mclip["i"][rb]),
                    ("u1", rm_u, mclip["r"][rb]), ("u2", im_u, mclip["i"][rb]),
                ):
                    t = stats.tile([128, 1], F32, name=f"sc{nm}{rb}")
                    nc.vector.tensor_scalar_mul(out=t[:], in0=m[:],
                                                scalar1=const / 127.0)
                    scl[nm] = t
                sg[rb] = scl

            with tc.tile_pool(name=f"wp{half}", bufs=4) as wpool, \
                 tc.tile_pool(name=f"ln{half}", bufs=2) as lnpool, \
                 tc.tile_pool(name=f"gu{half}", bufs=2) as gpool, \
                 tc.tile_pool(name=f"sp{half}", bufs=2) as spool, \
                 tc.tile_pool(name=f"tsp{half}", bufs=3) as tpool, \
                 tc.tile_pool(name=f"ps{half}", bufs=1, space="PSUM") as psp:
                for it, (n0, w) in enumerate(IM_TILES):
                    wt = {}
                    for lay in ("g", "u"):
                        for key in ("r", "i", "s"):
                            t = wpool.tile([128, KO, P1_W], F16, tag=f"w{key}",
                                           name=f"w{lay}{key}_{half}_{it}")
                            nc.sync.dma_start(t[:, :, :w], wg[lay + key][it].ap())
                            wt[lay + key] = t
                    lnr_t = lnpool.tile([128, P1_W], F32, tag="lnr",
                                        name=f"lnr{half}_{it}")
                    nc.gpsimd.dma_start(lnr_t[:, :w], ln_bcast(lnr_d, n0, w))
                    lni_t = lnpool.tile([128, P1_W], F32, tag="lni",
                                        name=f"lni{half}_{it}")
                    nc.gpsimd.dma_start(lni_t[:, :w], ln_bcast(lni_d, n0, w))

                    for rb in rbs:
                        rr = rb * 128
                        ps = {k: psp.tile([128, P1_W], F32, tag=f"ps{k}",
                                          name=f"ps{k}_{it}_{rb}")
                              for k in ("g1", "g2", "g3", "u1", "u2", "u3")}
                        for ko in range(KO):
                            st = ko == 0
                            sp = ko == KO - 1
                            lr = xqt["r"][rb][:, ko, :]
                            li = xqt["i"][rb][:, ko, :]
                            ls = xqt["s"][rb][:, ko, :]
                            nc.tensor.matmul(ps["g1"][:, :w], lr,
                                             wt["gr"][:, ko, :w], start=st, stop=sp)
                            nc.tensor.matmul(ps["u1"][:, :w], lr,
                                             wt["ur"][:, ko, :w], start=st, stop=sp)
                            nc.tensor.matmul(ps["g2"][:, :w], li,
                                             wt["gi"][:, ko, :w], start=st, stop=sp)
                            nc.tensor.matmul(ps["u2"][:, :w], li,
                                             wt["ui"][:, ko, :w], start=st, stop=sp)
                            nc.tensor.matmul(ps["g3"][:, :w], ls,
                                             wt["gs"][:, ko, :w], start=st, stop=sp)
                            nc.tensor.matmul(ps["u3"][:, :w], ls,
                                             wt["us"][:, ko, :w], start=st, stop=sp)

                        scl = sg[rb]
                        a1g = spool.tile([128, P1_W], F32, tag="a1g",
                                         name=f"a1g{it}_{rb}")
                        nc.scalar.activation(a1g[:, :w], ps["g1"][:, :w],
                                             Act.Copy, scale=scl["g1"][:])
                        a2g = spool.tile([128, P1_W], F32, tag="a2g",
                                         name=f"a2g{it}_{rb}")
                        nc.scalar.activation(a2g[:, :w], ps["g2"][:, :w],
                                             Act.Copy, scale=scl["g2"][:])
                        g_r = gpool.tile([128, P1_W], F32, tag="g_r",
                                         name=f"g_r{it}_{rb}")
                        nc.vector.tensor_add(out=g_r[:, :w], in0=a1g[:, :w],
                                             in1=a2g[:, :w])
                        tg = spool.tile([128, P1_W], F32, tag="tg",
                                        name=f"tg{it}_{rb}")
                        nc.vector.tensor_add(out=tg[:, :w], in0=ps["g3"][:, :w],
                                             in1=a2g[:, :w])
                        g_i = gpool.tile([128, P1_W], F32, tag="g_i",
                                         name=f"g_i{it}_{rb}")
                        nc.vector.tensor_sub(out=g_i[:, :w], in0=tg[:, :w],
                                             in1=a1g[:, :w])
                        a1u = spool.tile([128, P1_W], F32, tag="a1u",
                                         name=f"a1u{it}_{rb}")
                        nc.scalar.activation(a1u[:, :w], ps["u1"][:, :w],
                                             Act.Copy, scale=scl["u1"][:])
                        a2u = spool.tile([128, P1_W], F32, tag="a2u",
                                         name=f"a2u{it}_{rb}")
                        nc.scalar.activation(a2u[:, :w], ps["u2"][:, :w],
                                             Act.Copy, scale=scl["u2"][:])
                        u_r = gpool.tile([128, P1_W], F32, tag="u_r",
                                         name=f"u_r{it}_{rb}")
                        nc.vector.tensor_add(out=u_r[:, :w], in0=a1u[:, :w],
                                             in1=a2u[:, :w])
                        tu = spool.tile([128, P1_W], F32, tag="tu",
                                        name=f"tu{it}_{rb}")
                        nc.vector.tensor_add(out=tu[:, :w], in0=ps["u3"][:, :w],
                                             in1=a2u[:, :w])
                        u_i = gpool.tile([128, P1_W], F32, tag="u_i",
                                         name=f"u_i{it}_{rb}")
                        nc.vector.tensor_sub(out=u_i[:, :w], in0=tu[:, :w],
                                             in1=a1u[:, :w])

                        cm = spool.tile([128, P1_W], F32, tag="cm",
                                        name=f"cm{it}_{rb}")
                        nc.vector.tensor_max(out=cm[:, :w], in0=g_r[:, :w],
                                             in1=g_i[:, :w])
                        nc.vector.tensor_scalar(
                            out=cm[:, :w], in0=cm[:, :w], scalar1=0.0,
                            scalar2=None, op0=Alu.is_ge, op1=Alu.bypass)
                        p2 = spool.tile([128, P1_W], F32, tag="p2",
                                        name=f"p2{it}_{rb}")
                        nc.scalar.activation(p2[:, :w], g_r[:, :w], Act.Square)
                        q2 = spool.tile([128, P1_W], F32, tag="q2",
                                        name=f"q2{it}_{rb}")
                        nc.scalar.activation(q2[:, :w], g_i[:, :w], Act.Square)
                        m1 = spool.tile([128, P1_W], F32, tag="m1",
                                        name=f"m1{it}_{rb}")
                        nc.vector.tensor_mul(out=m1[:, :w], in0=p2[:, :w],
                                             in1=u_r[:, :w])
                        m2 = spool.tile([128, P1_W], F32, tag="m2",
                                        name=f"m2{it}_{rb}")
                        nc.vector.tensor_mul(out=m2[:, :w], in0=q2[:, :w],
                                             in1=u_i[:, :w])
                        h_r = spool.tile([128, P1_W], F32, tag="h_r",
                                         name=f"h_r{it}_{rb}")
                        nc.vector.tensor_add(out=h_r[:, :w], in0=m1[:, :w],
                                             in1=m2[:, :w])
                        nc.vector.tensor_mul(out=h_r[:, :w], in0=h_r[:, :w],
                                             in1=cm[:, :w])
                        m3 = spool.tile([128, P1_W], F32, tag="m3",
                                        name=f"m3{it}_{rb}")
                        nc.vector.tensor_mul(out=m3[:, :w], in0=p2[:, :w],
                                             in1=u_i[:, :w])
                        m4 = spool.tile([128, P1_W], F32, tag="m4",
                                        name=f"m4{it}_{rb}")
                        nc.vector.tensor_mul(out=m4[:, :w], in0=q2[:, :w],
                                             in1=u_r[:, :w])
                        h_i = spool.tile([128, P1_W], F32, tag="h_i",
                                         name=f"h_i{it}_{rb}")
                        nc.vector.tensor_sub(out=h_i[:, :w], in0=m3[:, :w],
                                             in1=m4[:, :w])
                        nc.vector.tensor_mul(out=h_i[:, :w], in0=h_i[:, :w],
                                             in1=cm[:, :w])

                        sq = spool.tile([128, P1_W], F32, tag="sq",
                                        name=f"sq{it}_{rb}")
                        nc.scalar.activation(sq[:, :w], h_r[:, :w], Act.Square)
                        ssp = spool.tile([128, 1], F32, tag="ssp",
                                         name=f"ssp{it}_{rb}")
                        nc.vector.tensor_reduce(
                            out=ssp[:], in_=sq[:, :w], axis=mybir.AxisListType.X,
                            op=Alu.add)
                        if it == 0:
                            nc.vector.tensor_copy(out=acc["ssr"][rb][0][:],
                                                  in_=ssp[:])
                        else:
                            nc.vector.tensor_add(
                                out=acc["ssr"][rb][it % 2][:],
                                in0=acc["ssr"][rb][(it - 1) % 2][:], in1=ssp[:])
                        t_r = tpool.tile([128, P1_W], F16, tag="t_r",
                                         name=f"t_r{it}_{rb}")
                        nc.vector.tensor_mul(out=t_r[:, :w], in0=h_r[:, :w],
                                             in1=lnr_t[:, :w])
                        amp = spool.tile([128, 1], F32, tag="amp",
                                         name=f"amp{it}_{rb}")
                        nc.vector.tensor_reduce(
                            out=amp[:], in_=t_r[:, :w], axis=mybir.AxisListType.X,
                            op=Alu.max, apply_absolute_value=True)
                        if it == 0:
                            nc.vector.tensor_scalar_max(
                                out=acc["amr"][rb][0][:], in0=amp[:], scalar1=0.0)
                        else:
                            nc.vector.tensor_tensor(
                                out=acc["amr"][rb][it % 2][:],
                                in0=acc["amr"][rb][(it - 1) % 2][:],
                                in1=amp[:], op=Alu.max)
                        sq2 = spool.tile([128, P1_W], F32, tag="sq2",
                                         name=f"sq2{it}_{rb}")
                        nc.scalar.activation(sq2[:, :w], h_i[:, :w], Act.Square)
                        ssp2 = spool.tile([128, 1], F32, tag="ssp2",
                                          name=f"ssp2{it}_{rb}")
                        nc.vector.tensor_reduce(
                            out=ssp2[:], in_=sq2[:, :w], axis=mybir.AxisListType.X,
                            op=Alu.add)
                        if it == 0:
                            nc.vector.tensor_copy(out=acc["ssi"][rb][0][:],
                                                  in_=ssp2[:])
                        else:
                            nc.vector.tensor_add(
                                out=acc["ssi"][rb][it % 2][:],
                                in0=acc["ssi"][rb][(it - 1) % 2][:], in1=ssp2[:])
                        t_i = tpool.tile([128, P1_W], F16, tag="t_i",
                                         name=f"t_i{it}_{rb}")
                        nc.vector.tensor_mul(out=t_i[:, :w], in0=h_i[:, :w],
                                             in1=lni_t[:, :w])
                        amp2 = spool.tile([128, 1], F32, tag="amp2",
                                          name=f"amp2{it}_{rb}")
                        nc.vector.tensor_reduce(
                            out=amp2[:], in_=t_i[:, :w], axis=mybir.AxisListType.X,
                            op=Alu.max, apply_absolute_value=True)
                        if it == 0:
                            nc.vector.tensor_scalar_max(
                                out=acc["ami"][rb][0][:], in0=amp2[:], scalar1=0.0)
                        else:
                            nc.vector.tensor_tensor(
                                out=acc["ami"][rb][it % 2][:],
                                in0=acc["ami"][rb][(it - 1) % 2][:],
                                in1=amp2[:], op=Alu.max)
                        nc.gpsimd.dma_start(tr_d.ap()[rr:rr + 128, n0:n0 + w],
                                            t_r[:, :w])
                        nc.gpsimd.dma_start(ti_d.ap()[rr:rr + 128, n0:n0 + w],
                                            t_i[:, :w])

          # ---------------- phase 2 + 3 for this half ----------------
          with tc.tile_pool(name=f"nqt{half}", bufs=1) as nqt_pool, \
               tc.tile_pool(name=f"p2s{half}", bufs=2) as p2s, \
               tc.tile_pool(name=f"dt{half}", bufs=2) as dtpool, \
               tc.tile_pool(name=f"oo{half}", bufs=3) as opool, \
               tc.tile_pool(name=f"od{half}", bufs=2) as ospool, \
               tc.tile_pool(name=f"psd{half}", bufs=2, space="PSUM") as psd:
            nqt = {"r": {}, "i": {}}
            dsc = {}
            for rb in rbs:
                rr = rb * 128
                ssr = acc["ssr"][rb][last]
                ssi = acc["ssi"][rb][last]
                su = stats.tile([128, 1], F32, name=f"su{rb}")
                nc.vector.tensor_add(out=su[:], in0=ssr[:], in1=ssi[:])
                me = stats.tile([128, 1], F32, name=f"me{rb}")
                nc.vector.tensor_scalar(
                    out=me[:], in0=su[:], scalar1=1.0 / IM,
                    scalar2=EPS, op0=Alu.mult, op1=Alu.add)
                sr = stats.tile([128, 1], F32, name=f"sr{rb}")
                nc.scalar.activation(sr[:], me[:], Act.Sqrt)
                inv0 = stats.tile([128, 1], F32, name=f"inv0{rb}")
                nc.vector.reciprocal(inv0[:], sr[:])
                nw = stats.tile([128, 1], F32, name=f"nw{rb}")
                nc.vector.tensor_mul(out=nw[:], in0=inv0[:], in1=inv0[:])
                nc.vector.tensor_mul(out=nw[:], in0=nw[:], in1=me[:])
                nc.vector.tensor_scalar(
                    out=nw[:], in0=nw[:], scalar1=-0.5, scalar2=1.5,
                    op0=Alu.mult, op1=Alu.add)
                inv = stats.tile([128, 1], F32, name=f"inv{rb}")
                nc.vector.tensor_mul(out=inv[:], in0=inv0[:], in1=nw[:])

                sc = {}
                for comp, amk in (("r", "amr"), ("i", "ami")):
                    am = acc[amk][rb][last]
                    amn = stats.tile([128, 1], F32, name=f"amn{comp}{rb}")
                    nc.vector.tensor_mul(out=amn[:], in0=am[:], in1=inv[:])
                    nc.vector.tensor_scalar_max(out=amn[:], in0=amn[:],
                                                scalar1=1e-5)
                    rsn = stats.tile([128, 1], F32, name=f"rsn{comp}{rb}")
                    nc.vector.reciprocal(rsn[:], amn[:])
                    nc.vector.tensor_scalar_mul(out=rsn[:], in0=rsn[:],
                                                scalar1=127.0)
                    cq = stats.tile([128, 1], F32, name=f"cq{comp}{rb}")
                    nc.vector.tensor_mul(out=cq[:], in0=inv[:], in1=rsn[:])
                    sc[f"cq{comp}"] = cq
                    sc[f"amn{comp}"] = amn
                for nm, const, amn in (
                    ("d1", rm_d, sc["amnr"]), ("d2", im_d, sc["amni"]),
                    ("d3", im_d, sc["amnr"]), ("d4", rm_d, sc["amni"]),
                ):
                    t = stats.tile([128, 1], F32, name=f"ds{nm}{rb}")
                    nc.vector.tensor_scalar_mul(out=t[:], in0=amn[:],
                                                scalar1=const / 127.0)
                    sc[nm] = t
                dsc[rb] = sc

                for comp, t_d in (("r", tr_d), ("i", ti_d)):
                    teng = nc.sync if comp == "r" else nc.gpsimd
                    nqt_t = nqt_pool.tile([128, IO, 128], F16,
                                          name=f"nqt{comp}{rb}")
                    for ci, (io0, nio) in enumerate(QCH):
                        cw = nio * 128
                        c0 = io0 * 128
                        tin = p2s.tile([128, 7 * 128], F16, tag="tin",
                                       name=f"tin{comp}{rb}_{ci}")
                        teng.dma_start(
                            tin[:, :cw], t_d.ap()[rr:rr + 128, c0:c0 + cw])
                        s1 = p2s.tile([128, 7 * 128], F32, tag="s1q",
                                      name=f"s1q{comp}{rb}_{ci}")
                        nc.scalar.activation(s1[:, :cw], tin[:, :cw],
                                             Act.Copy, bias=MAGIC,
                                             scale=sc[f"cq{comp}"][:])
                        nq = p2s.tile([128, 7 * 128], F16, tag="nq",
                                      name=f"nq{comp}{rb}_{ci}")
                        nc.vector.tensor_scalar(
                            out=nq[:, :cw], in0=s1[:, :cw], scalar1=MAGIC,
                            scalar2=None, op0=Alu.subtract, op1=Alu.bypass)
                        nc.scalar.dma_start_transpose(
                            nqt_t[:, io0:io0 + nio, :], nq[:, :cw])
                    nqt[comp][rb] = nqt_t

            for ht, (h0, hw) in enumerate(HID_TILES):
                DCH = [(0, 11), (11, 11), (22, 11), (33, 10)]
                dtr = dtpool.tile([128, IO, HT_W], F16, tag="dtr",
                                  name=f"dtr{half}_{ht}")
                for qi, (i0, ni) in enumerate(DCH):
                    eng = nc.sync if qi % 2 == 0 else nc.gpsimd
                    eng.dma_start(dtr[:, i0:i0 + ni, :],
                                  wd["dr"][ht].ap()[:, i0:i0 + ni, :])
                dti = dtpool.tile([128, IO, HT_W], F16, tag="dti",
                                  name=f"dti{half}_{ht}")
                for qi, (i0, ni) in enumerate(DCH):
                    eng = nc.sync if qi % 2 == 0 else nc.gpsimd
                    eng.dma_start(dti[:, i0:i0 + ni, :],
                                  wd["di"][ht].ap()[:, i0:i0 + ni, :])
                for rb in rbs:
                    rr = rb * 128
                    pd = {k: psd.tile([128, HT_W], F32, tag=f"pd{k}",
                                      name=f"pd{k}_{half}_{ht}_{rb}")
                          for k in ("1", "2", "3", "4")}
                    for io in range(IO):
                        st = io == 0
                        sp = io == IO - 1
                        lr = nqt["r"][rb][:, io, :]
                        li = nqt["i"][rb][:, io, :]
                        nc.tensor.matmul(pd["1"][:], lr, dtr[:, io, :],
                                         start=st, stop=sp)
                        nc.tensor.matmul(pd["3"][:], lr, dti[:, io, :],
                                         start=st, stop=sp)
                        nc.tensor.matmul(pd["2"][:], li, dti[:, io, :],
                                         start=st, stop=sp)
                        nc.tensor.matmul(pd["4"][:], li, dtr[:, io, :],
                                         start=st, stop=sp)
                    sc = dsc[rb]
                    ob = ospool.tile([128, HT_W], F32, tag="ob",
                                     name=f"ob{half}_{ht}_{rb}")
                    nc.scalar.activation(ob[:], pd["2"][:], Act.Copy,
                                         scale=sc["d2"][:])
                    o_r = opool.tile([128, HT_W], F32, tag="o_r",
                                     name=f"o_r{half}_{ht}_{rb}")
                    nc.vector.scalar_tensor_tensor(
                        out=o_r[:], in0=pd["1"][:], scalar=sc["d1"][:],
                        in1=ob[:], op0=Alu.mult, op1=Alu.add)
                    nc.gpsimd.dma_start(
                        or_d.ap()[rr:rr + 128, h0:h0 + hw], o_r[:])
                    ob2 = ospool.tile([128, HT_W], F32, tag="ob2",
                                      name=f"ob2{half}_{ht}_{rb}")
                    nc.scalar.activation(ob2[:], pd["4"][:], Act.Copy,
                                         scale=sc["d4"][:])
                    o_i = opool.tile([128, HT_W], F32, tag="o_i",
                                     name=f"o_i{half}_{ht}_{rb}")
                    nc.vector.scalar_tensor_tensor(
                        out=o_i[:], in0=pd["3"][:], scalar=sc["d3"][:],
                        in1=ob2[:], op0=Alu.mult, op1=Alu.subtract)
                    nc.gpsimd.dma_start(
                        oi_d.ap()[rr:rr + 128, h0:h0 + hw], o_i[:])

    nc.compile()
    return nc


_CACHE = {}


def _get_program(key):
    if key not in _CACHE:
        _CACHE[key] = _build_program(*key)
    return _CACHE[key]


_WCACHE = {}


def kernel(x_real, x_imag, gate_wr, gate_wi, up_wr, up_wi,
           down_wr, down_wi, ln_wr, ln_wi, **run_kwargs):
    tgr, tgi, rm_g, im_g = _weight_prep(np.asarray(gate_wr), np.asarray(gate_wi))
    tur, tui, rm_u, im_u = _weight_prep(np.asarray(up_wr), np.asarray(up_wi))
    tdr, tdi, rm_d, im_d = _weight_prep(np.asarray(down_wr), np.asarray(down_wi))
    key = (rm_g, im_g, rm_u, im_u, rm_d, im_d)

    nc = _get_program(key)

    if key not in _WCACHE:
        shared = {}
        for k, tiles in (
            ("gr", _pack_p1(tgr)), ("gi", _pack_p1(tgi)),
            ("gs", _pack_p1(tgr * rm_g + tgi * im_g)),
            ("ur", _pack_p1(tur)), ("ui", _pack_p1(tui)),
            ("us", _pack_p1(tur * rm_u + tui * im_u)),
        ):
            for it, arr in enumerate(tiles):
                shared[f"w{k}_{it}"] = arr
        for k, tiles in (("dr", _pack_dn(tdr)), ("di", _pack_dn(tdi))):
            for ht, arr in enumerate(tiles):
                shared[f"w{k}_{ht}"] = arr
        shared["ln_r"] = np.asarray(ln_wr, np.float32).reshape(1, IM)
        shared["ln_i"] = np.asarray(ln_wi, np.float32).reshape(1, IM)
        _WCACHE.clear()
        _WCACHE[key] = shared
    shared = _WCACHE[key]

    xr = np.ascontiguousarray(np.asarray(x_real, np.float32).reshape(ROWS, HIDDEN))
    xi = np.ascontiguousarray(np.asarray(x_imag, np.float32).reshape(ROWS, HIDDEN))

    in_maps = []
    for c in range(NCORES):
        sl = slice(c * RPC, (c + 1) * RPC)
        in_maps.append({"xr": np.ascontiguousarray(xr[sl]),
                        "xi": np.ascontiguousarray(xi[sl]), **shared})

    res = run_bass_kernel_spmd(nc, in_maps, core_ids=list(range(NCORES)),
                               **run_kwargs)

    out_r = np.concatenate([res.results[c]["o_r"] for c in range(NCORES)],
                           axis=0).reshape(B, S, HIDDEN)
    out_i = np.concatenate([res.results[c]["o_i"] for c in range(NCORES)],
                           axis=0).reshape(B, S, HIDDEN)
    kernel.last_results = res
    return out_r, out_i


# revision 16
# speedup vs baseline: 1.0061x; 1.0061x over previous
"""Trainium2 Bass kernel for nn_ComplexNetMLP (complex ternary-quant MLP).

Data-parallel over the 8192 rows across 8 NeuronCores (1024 rows each, no
collectives). Strategy vs the earlier version:

- Gate/up complex linears use a 3-matmul Karatsuba decomposition
  (t1 = qxr@qwr.T, t2 = qxi@qwi.T, t3 = (qxr-qxi)@(qwr+qwi).T;
   out_r = t1+t2, out_i = t3-t1+t2). t1/t2 keep exact int8-in-fp16
  operands against exact ternary weights; only the t3 chain uses
  fp16-rounded combined operands (keeps added rel-err ~5e-3, validated
  against the reference in simulation). Cuts gate/up PE work by 25%.
- fp16 on the whole matmul path (same PE rate as bf16, 8x finer mantissa).
- Host-packed per-tile weight layouts: one DMA per weight tile with
  8-22KB contiguous per-partition lines (instead of 16-43 narrow DMAs).
- Epilogue element-wise work spread across DVE + Pool + Act engines
  (Pool was idle before); DMA issue spread across SP/Act/Pool queues.
- The h*ln spill to DRAM is fp16: half the reload traffic on the
  phase-1 -> phase-3 transition, which was a 140us PE bubble.
"""

import sys

sys.path.insert(0, "/opt/trn_rl_repo")

import numpy as np

import concourse.bass as bass
import concourse.tile as tile
from concourse import bacc, mybir
from concourse.bass_utils import run_bass_kernel_spmd

F32 = mybir.dt.float32
F16 = mybir.dt.float16

HIDDEN = 2048
IM = 5504
B, S = 4, 2048
ROWS = B * S              # 8192
NCORES = 8
RPC = ROWS // NCORES      # 1024 rows per core
NRB = RPC // 128          # 8 row-blocks
NHALF = 2
RBH = NRB // NHALF        # 4 row-blocks per half
KO = HIDDEN // 128        # 16 k-chunks for gate/up
IO = IM // 128            # 43 im-chunks for down
EPS = 1e-6
MAGIC = float(np.float32(12582912.0))  # 1.5 * 2**23, RNE rounding magic

P1_W = 256
IM_TILES = [(i * P1_W, min(P1_W, IM - i * P1_W)) for i in range((IM + P1_W - 1) // P1_W)]
NIT = len(IM_TILES)        # 22
HT_W = 256
HID_TILES = [(i * HT_W, HT_W) for i in range(HIDDEN // HT_W)]
NHT = len(HID_TILES)       # 8

Alu = mybir.AluOpType
Act = mybir.ActivationFunctionType


def _weight_prep(wr, wi):
    """Ternary sign matrices + scales (same math as reference.weight_quant)."""
    awr = np.abs(wr)
    awi = np.abs(wi)
    rmask = awr > awi
    imask = ~rmask
    tr = np.where(rmask, np.sign(wr), 0.0).astype(np.float64)
    ti = np.where(imask, np.sign(wi), 0.0).astype(np.float64)
    rcnt = np.maximum(rmask.sum(dtype=np.float64), 1.0)
    icnt = np.maximum(imask.sum(dtype=np.float64), 1.0)
    rmean = np.float32(np.sum(awr * rmask, dtype=np.float64) / rcnt)
    imean = np.float32(np.sum(awi * imask, dtype=np.float64) / icnt)
    rm = float(np.clip(rmean, np.float32(1e-5), None))
    im = float(np.clip(imean, np.float32(1e-5), None))
    return tr, ti, rm, im


def _pack_p1(mat):
    """[IM, HIDDEN] f64 -> per-im-tile [128, KO, w] f16 (ki on partitions)."""
    a = mat.T  # [HIDDEN, IM]
    out = []
    for n0, w in IM_TILES:
        t = a[:, n0:n0 + w].reshape(KO, 128, w).transpose(1, 0, 2)
        out.append(np.ascontiguousarray(t).astype(np.float16))
    return out


def _pack_dn(mat):
    """[HIDDEN, IM] f64 -> per-hid-tile [128, IO, 256] f16 (ii on partitions)."""
    a = mat.T  # [IM, HIDDEN]
    out = []
    for h0, hw in HID_TILES:
        t = a[:, h0:h0 + hw].reshape(IO, 128, hw).transpose(1, 0, 2)
        out.append(np.ascontiguousarray(t).astype(np.float16))
    return out


def _build_program(rm_g, im_g, rm_u, im_u, rm_d, im_d):
    nc = bacc.Bacc("TRN2", target_bir_lowering=False, debug=False,
                   num_devices=NCORES)

    xr_d = nc.dram_tensor("xr", [RPC, HIDDEN], F32, kind="ExternalInput")
    xi_d = nc.dram_tensor("xi", [RPC, HIDDEN], F32, kind="ExternalInput")
    wg = {}
    for key in ("gr", "gi", "gs", "ur", "ui", "us"):
        wg[key] = [nc.dram_tensor(f"w{key}_{it}", [128, KO, w], F16,
                                  kind="ExternalInput")
                   for it, (n0, w) in enumerate(IM_TILES)]
    wd = {}
    for key in ("dr", "di"):
        wd[key] = [nc.dram_tensor(f"w{key}_{ht}", [128, IO, HT_W], F16,
                                  kind="ExternalInput")
                   for ht in range(NHT)]
    lnr_d = nc.dram_tensor("ln_r", [1, IM], F32, kind="ExternalInput")
    lni_d = nc.dram_tensor("ln_i", [1, IM], F32, kind="ExternalInput")
    or_d = nc.dram_tensor("o_r", [RPC, HIDDEN], F32, kind="ExternalOutput")
    oi_d = nc.dram_tensor("o_i", [RPC, HIDDEN], F32, kind="ExternalOutput")

    tr_d = nc.dram_tensor("t_r_scratch", [RPC, IM], F16)
    ti_d = nc.dram_tensor("t_i_scratch", [RPC, IM], F16)

    def ln_bcast(handle, n0, w):
        return bass.AP(tensor=handle, offset=n0, ap=[[0, 128], [1, w]])

    # phase-2 quant io-chunking (43 io chunks of 128 -> 7 chunks)
    QCH = [(0, 6), (6, 6), (12, 6), (18, 6), (24, 6), (30, 6), (36, 7)]

    with tile.TileContext(nc) as tc:
      with tc.tile_pool(name="stats", bufs=1) as stats:
        acc = {k: [[stats.tile([128, 1], F32, name=f"acc{k}{rb}{s}")
                    for s in range(2)] for rb in range(NRB)]
               for k in ("ssr", "ssi", "amr", "ami")}
        last = (NIT - 1) % 2

        for half in range(NHALF):
          rbs = [half * RBH + i for i in range(RBH)]

          # ---------------- prologue + phase 1 ----------------
          with tc.tile_pool(name=f"xqt{half}", bufs=1) as xqt_pool, \
               tc.tile_pool(name=f"wp{half}", bufs=4) as wpool:

            def load_w(lay, key, it):
                t = wpool.tile([128, KO, P1_W], F16, tag=f"w{key}",
                               name=f"w{lay}{key}_{half}_{it}")
                nc.sync.dma_start(t[:, :, :IM_TILES[it][1]],
                                  wg[lay + key][it].ap())
                return t

            wt0 = {"gr": load_w("g", "r", 0)}
            xqt = {"r": {}, "i": {}, "s": {}}
            sg = {}
            mclip = {"r": {}, "i": {}}
            for rb in rbs:
                rr = rb * 128
                with tc.tile_pool(name=f"prol{rb}", bufs=1) as pp:
                    c1t = {}
                    for comp, x_d in (("r", xr_d), ("i", xi_d)):
                        xt = pp.tile([128, HIDDEN], F32, name=f"x{comp}{rb}")
                        nc.scalar.dma_start(xt[:], x_d.ap()[rr:rr + 128, :])
                        am = stats.tile([128, 1], F32, name=f"am{comp}{rb}")
                        nc.vector.tensor_reduce(
                            out=am[:], in_=xt[:], axis=mybir.AxisListType.X,
                            op=Alu.max, apply_absolute_value=True)
                        m = stats.tile([128, 1], F32, name=f"m{comp}{rb}")
                        nc.vector.tensor_scalar_max(out=m[:], in0=am[:],
                                                    scalar1=1e-5)
                        mclip[comp][rb] = m
                        rcp = stats.tile([128, 1], F32, name=f"rcp{comp}{rb}")
                        nc.vector.reciprocal(rcp[:], m[:])
                        c = stats.tile([128, 1], F32, name=f"c{comp}{rb}")
                        nc.vector.tensor_scalar_mul(out=c[:], in0=rcp[:],
                                                    scalar1=127.0)
                        dl = stats.tile([128, 1], F32, name=f"dl{comp}{rb}")
                        nc.vector.tensor_scalar_mul(out=dl[:], in0=m[:],
                                                    scalar1=1.0 / 127.0)
                        nc.vector.tensor_scalar(
                            out=xt[:], in0=xt[:], scalar1=c[:],
                            scalar2=MAGIC, op0=Alu.mult, op1=Alu.add)
                        xq = pp.tile([128, HIDDEN], F16, name=f"xq{comp}{rb}")
                        nc.scalar.activation(xq[:], xt[:], Act.Copy,
                                             bias=-MAGIC)
                        xqt_t = xqt_pool.tile([128, KO, 128], F16,
                                              name=f"xqt{comp}{rb}")
                        nc.sync.dma_start_transpose(xqt_t[:], xq[:])
                        xqt[comp][rb] = xqt_t
                        c1 = pp.tile([128, HIDDEN], F32, name=f"c1{comp}{rb}")
                        nc.scalar.activation(c1[:], xq[:], Act.Copy,
                                             scale=dl[:])
                        c1t[comp] = c1
                    b3 = pp.tile([128, HIDDEN], F16, name=f"b3{rb}")
                    nc.vector.tensor_sub(out=b3[:], in0=c1t["r"][:],
                                         in1=c1t["i"][:])
                    b3t = xqt_pool.tile([128, KO, 128], F16, name=f"b3t{rb}")
                    nc.sync.dma_start_transpose(b3t[:], b3[:])
                    xqt["s"][rb] = b3t
                scl = {}
                for nm, const, m in (
                    ("g1", rm_g, mclip["r"][rb]), ("g2", im_g, mclip["i"][rb]),
                    ("u1", rm_u, mclip["r"][rb]), ("u2", im_u, mclip["i"][rb]),
                ):
                    t = stats.tile([128, 1], F32, name=f"sc{nm}{rb}")
                    nc.vector.tensor_scalar_mul(out=t[:], in0=m[:],
                                                scalar1=const / 127.0)
                    scl[nm] = t
                sg[rb] = scl

            for lay, key in (("g", "i"), ("g", "s"), ("u", "r"),
                             ("u", "i"), ("u", "s")):
                wt0[lay + key] = load_w(lay, key, 0)

            with tc.tile_pool(name=f"ln{half}", bufs=2) as lnpool, \
                 tc.tile_pool(name=f"gu{half}", bufs=2) as gpool, \
                 tc.tile_pool(name=f"sp{half}", bufs=2) as spool, \
                 tc.tile_pool(name=f"tsp{half}", bufs=3) as tpool, \
                 tc.tile_pool(name=f"ps{half}", bufs=1, space="PSUM") as psp:
                for it, (n0, w) in enumerate(IM_TILES):
                    if it == 0:
                        wt = wt0
                    else:
                        wt = {lay + key: load_w(lay, key, it)
                              for lay in ("g", "u") for key in ("r", "i", "s")}
                    lnr_t = lnpool.tile([128, P1_W], F32, tag="lnr",
                                        name=f"lnr{half}_{it}")
                    nc.sync.dma_start(lnr_t[:, :w], ln_bcast(lnr_d, n0, w))
                    lni_t = lnpool.tile([128, P1_W], F32, tag="lni",
                                        name=f"lni{half}_{it}")
                    nc.sync.dma_start(lni_t[:, :w], ln_bcast(lni_d, n0, w))

                    for rb in rbs:
                        rr = rb * 128
                        ps = {k: psp.tile([128, P1_W], F32, tag=f"ps{k}",
                                          name=f"ps{k}_{it}_{rb}")
                              for k in ("g1", "g2", "g3", "u1", "u2", "u3")}
                        for ko in range(KO):
                            st = ko == 0
                            sp = ko == KO - 1
                            lr = xqt["r"][rb][:, ko, :]
                            li = xqt["i"][rb][:, ko, :]
                            ls = xqt["s"][rb][:, ko, :]
                            nc.tensor.matmul(ps["g1"][:, :w], lr,
                                             wt["gr"][:, ko, :w], start=st, stop=sp)
                            nc.tensor.matmul(ps["u1"][:, :w], lr,
                                             wt["ur"][:, ko, :w], start=st, stop=sp)
                            nc.tensor.matmul(ps["g2"][:, :w], li,
                                             wt["gi"][:, ko, :w], start=st, stop=sp)
                            nc.tensor.matmul(ps["u2"][:, :w], li,
                                             wt["ui"][:, ko, :w], start=st, stop=sp)
                            nc.tensor.matmul(ps["g3"][:, :w], ls,
                                             wt["gs"][:, ko, :w], start=st, stop=sp)
                            nc.tensor.matmul(ps["u3"][:, :w], ls,
                                             wt["us"][:, ko, :w], start=st, stop=sp)

                        scl = sg[rb]
                        a1g = spool.tile([128, P1_W], F32, tag="a1g",
                                         name=f"a1g{it}_{rb}")
                        nc.scalar.activation(a1g[:, :w], ps["g1"][:, :w],
                                             Act.Copy, scale=scl["g1"][:])
                        a2g = spool.tile([128, P1_W], F32, tag="a2g",
                                         name=f"a2g{it}_{rb}")
                        nc.scalar.activation(a2g[:, :w], ps["g2"][:, :w],
                                             Act.Copy, scale=scl["g2"][:])
                        g_r = gpool.tile([128, P1_W], F32, tag="g_r",
                                         name=f"g_r{it}_{rb}")
                        nc.vector.tensor_add(out=g_r[:, :w], in0=a1g[:, :w],
                                             in1=a2g[:, :w])
                        tg = spool.tile([128, P1_W], F32, tag="tg",
                                        name=f"tg{it}_{rb}")
                        nc.vector.tensor_add(out=tg[:, :w], in0=ps["g3"][:, :w],
                                             in1=a2g[:, :w])
                        g_i = gpool.tile([128, P1_W], F32, tag="g_i",
                                         name=f"g_i{it}_{rb}")
                        nc.vector.tensor_sub(out=g_i[:, :w], in0=tg[:, :w],
                                             in1=a1g[:, :w])
                        a1u = spool.tile([128, P1_W], F32, tag="a1u",
                                         name=f"a1u{it}_{rb}")
                        nc.scalar.activation(a1u[:, :w], ps["u1"][:, :w],
                                             Act.Copy, scale=scl["u1"][:])
                        a2u = spool.tile([128, P1_W], F32, tag="a2u",
                                         name=f"a2u{it}_{rb}")
                        nc.scalar.activation(a2u[:, :w], ps["u2"][:, :w],
                                             Act.Copy, scale=scl["u2"][:])
                        u_r = gpool.tile([128, P1_W], F32, tag="u_r",
                                         name=f"u_r{it}_{rb}")
                        nc.vector.tensor_add(out=u_r[:, :w], in0=a1u[:, :w],
                                             in1=a2u[:, :w])
                        tu = spool.tile([128, P1_W], F32, tag="tu",
                                        name=f"tu{it}_{rb}")
                        nc.vector.tensor_add(out=tu[:, :w], in0=ps["u3"][:, :w],
                                             in1=a2u[:, :w])
                        u_i = gpool.tile([128, P1_W], F32, tag="u_i",
                                         name=f"u_i{it}_{rb}")
                        nc.vector.tensor_sub(out=u_i[:, :w], in0=tu[:, :w],
                                             in1=a1u[:, :w])

                        cm = spool.tile([128, P1_W], F32, tag="cm",
                                        name=f"cm{it}_{rb}")
                        nc.vector.tensor_max(out=cm[:, :w], in0=g_r[:, :w],
                                             in1=g_i[:, :w])
                        nc.vector.tensor_scalar(
                            out=cm[:, :w], in0=cm[:, :w], scalar1=0.0,
                            scalar2=None, op0=Alu.is_ge, op1=Alu.bypass)
                        p2 = spool.tile([128, P1_W], F32, tag="p2",
                                        name=f"p2{it}_{rb}")
                        nc.scalar.activation(p2[:, :w], g_r[:, :w], Act.Square)
                        q2 = spool.tile([128, P1_W], F32, tag="q2",
                                        name=f"q2{it}_{rb}")
                        nc.scalar.activation(q2[:, :w], g_i[:, :w], Act.Square)
                        m1 = spool.tile([128, P1_W], F32, tag="m1",
                                        name=f"m1{it}_{rb}")
                        nc.vector.tensor_mul(out=m1[:, :w], in0=p2[:, :w],
                                             in1=u_r[:, :w])
                        m2 = spool.tile([128, P1_W], F32, tag="m2",
                                        name=f"m2{it}_{rb}")
                        nc.vector.tensor_mul(out=m2[:, :w], in0=q2[:, :w],
                                             in1=u_i[:, :w])
                        h_r = spool.tile([128, P1_W], F32, tag="h_r",
                                         name=f"h_r{it}_{rb}")
                        nc.vector.tensor_add(out=h_r[:, :w], in0=m1[:, :w],
                                             in1=m2[:, :w])
                        nc.vector.tensor_mul(out=h_r[:, :w], in0=h_r[:, :w],
                                             in1=cm[:, :w])
                        m3 = spool.tile([128, P1_W], F32, tag="m3",
                                        name=f"m3{it}_{rb}")
                        nc.vector.tensor_mul(out=m3[:, :w], in0=p2[:, :w],
                                             in1=u_i[:, :w])
                        m4 = spool.tile([128, P1_W], F32, tag="m4",
                                        name=f"m4{it}_{rb}")
                        nc.vector.tensor_mul(out=m4[:, :w], in0=q2[:, :w],
                                             in1=u_r[:, :w])
                        h_i = spool.tile([128, P1_W], F32, tag="h_i",
                                         name=f"h_i{it}_{rb}")
                        nc.vector.tensor_sub(out=h_i[:, :w], in0=m3[:, :w],
                                             in1=m4[:, :w])
                        nc.vector.tensor_mul(out=h_i[:, :w], in0=h_i[:, :w],
                                             in1=cm[:, :w])

                        sq = spool.tile([128, P1_W], F32, tag="sq",
                                        name=f"sq{it}_{rb}")
                        nc.scalar.activation(sq[:, :w], h_r[:, :w], Act.Square)
                        ssp = spool.tile([128, 1], F32, tag="ssp",
                                         name=f"ssp{it}_{rb}")
                        nc.vector.tensor_reduce(
                            out=ssp[:], in_=sq[:, :w], axis=mybir.AxisListType.X,
                            op=Alu.add)
                        if it == 0:
                            nc.vector.tensor_copy(out=acc["ssr"][rb][0][:],
                                                  in_=ssp[:])
                        else:
                            nc.vector.tensor_add(
                                out=acc["ssr"][rb][it % 2][:],
                                in0=acc["ssr"][rb][(it - 1) % 2][:], in1=ssp[:])
                        t_r = tpool.tile([128, P1_W], F16, tag="t_r",
                                         name=f"t_r{it}_{rb}")
                        nc.vector.tensor_mul(out=t_r[:, :w], in0=h_r[:, :w],
                                             in1=lnr_t[:, :w])
                        amp = spool.tile([128, 1], F32, tag="amp",
                                         name=f"amp{it}_{rb}")
                        nc.vector.tensor_reduce(
                            out=amp[:], in_=t_r[:, :w], axis=mybir.AxisListType.X,
                            op=Alu.max, apply_absolute_value=True)
                        if it == 0:
                            nc.vector.tensor_scalar_max(
                                out=acc["amr"][rb][0][:], in0=amp[:], scalar1=0.0)
                        else:
                            nc.vector.tensor_tensor(
                                out=acc["amr"][rb][it % 2][:],
                                in0=acc["amr"][rb][(it - 1) % 2][:],
                                in1=amp[:], op=Alu.max)
                        sq2 = spool.tile([128, P1_W], F32, tag="sq2",
                                         name=f"sq2{it}_{rb}")
                        nc.scalar.activation(sq2[:, :w], h_i[:, :w], Act.Square)
                        ssp2 = spool.tile([128, 1], F32, tag="ssp2",
                                          name=f"ssp2{it}_{rb}")
                        nc.vector.tensor_reduce(
                            out=ssp2[:], in_=sq2[:, :w], axis=mybir.AxisListType.X,
                            op=Alu.add)
                        if it == 0:
                            nc.vector.tensor_copy(out=acc["ssi"][rb][0][:],
                                                  in_=ssp2[:])
                        else:
                            nc.vector.tensor_add(
                                out=acc["ssi"][rb][it % 2][:],
                                in0=acc["ssi"][rb][(it - 1) % 2][:], in1=ssp2[:])
                        t_i = tpool.tile([128, P1_W], F16, tag="t_i",
                                         name=f"t_i{it}_{rb}")
                        nc.vector.tensor_mul(out=t_i[:, :w], in0=h_i[:, :w],
                                             in1=lni_t[:, :w])
                        amp2 = spool.tile([128, 1], F32, tag="amp2",
                                          name=f"amp2{it}_{rb}")
                        nc.vector.tensor_reduce(
                            out=amp2[:], in_=t_i[:, :w], axis=mybir.AxisListType.X,
                            op=Alu.max, apply_absolute_value=True)
                        if it == 0:
                            nc.vector.tensor_scalar_max(
                                out=acc["ami"][rb][0][:], in0=amp2[:], scalar1=0.0)
                        else:
                            nc.vector.tensor_tensor(
                                out=acc["ami"][rb][it % 2][:],
                                in0=acc["ami"][rb][(it - 1) % 2][:],
                                in1=amp2[:], op=Alu.max)
                        nc.scalar.dma_start(tr_d.ap()[rr:rr + 128, n0:n0 + w],
                                            t_r[:, :w])
                        nc.scalar.dma_start(ti_d.ap()[rr:rr + 128, n0:n0 + w],
                                            t_i[:, :w])

          # ---------------- phase 2 + 3 for this half ----------------
          with tc.tile_pool(name=f"nqt{half}", bufs=1) as nqt_pool, \
               tc.tile_pool(name=f"p2s{half}", bufs=2) as p2s, \
               tc.tile_pool(name=f"dt{half}", bufs=2) as dtpool, \
               tc.tile_pool(name=f"oo{half}", bufs=3) as opool, \
               tc.tile_pool(name=f"od{half}", bufs=2) as ospool, \
               tc.tile_pool(name=f"psd{half}", bufs=2, space="PSUM") as psd:
            nqt = {"r": {}, "i": {}}
            dsc = {}

            def load_dt(ht):
                dtr = dtpool.tile([128, IO, HT_W], F16, tag="dtr",
                                  name=f"dtr{half}_{ht}")
                nc.sync.dma_start(dtr[:], wd["dr"][ht].ap())
                dti = dtpool.tile([128, IO, HT_W], F16, tag="dti",
                                  name=f"dti{half}_{ht}")
                nc.sync.dma_start(dti[:], wd["di"][ht].ap())
                return dtr, dti

            dtw = {}
            for ri, rb in enumerate(rbs):
                rr = rb * 128
                ssr = acc["ssr"][rb][last]
                ssi = acc["ssi"][rb][last]
                su = stats.tile([128, 1], F32, name=f"su{rb}")
                nc.vector.tensor_add(out=su[:], in0=ssr[:], in1=ssi[:])
                me = stats.tile([128, 1], F32, name=f"me{rb}")
                nc.vector.tensor_scalar(
                    out=me[:], in0=su[:], scalar1=1.0 / IM,
                    scalar2=EPS, op0=Alu.mult, op1=Alu.add)
                sr = stats.tile([128, 1], F32, name=f"sr{rb}")
                nc.scalar.activation(sr[:], me[:], Act.Sqrt)
                inv0 = stats.tile([128, 1], F32, name=f"inv0{rb}")
                nc.vector.reciprocal(inv0[:], sr[:])
                nw = stats.tile([128, 1], F32, name=f"nw{rb}")
                nc.vector.tensor_mul(out=nw[:], in0=inv0[:], in1=inv0[:])
                nc.vector.tensor_mul(out=nw[:], in0=nw[:], in1=me[:])
                nc.vector.tensor_scalar(
                    out=nw[:], in0=nw[:], scalar1=-0.5, scalar2=1.5,
                    op0=Alu.mult, op1=Alu.add)
                inv = stats.tile([128, 1], F32, name=f"inv{rb}")
                nc.vector.tensor_mul(out=inv[:], in0=inv0[:], in1=nw[:])

                sc = {}
                for comp, amk in (("r", "amr"), ("i", "ami")):
                    am = acc[amk][rb][last]
                    amn = stats.tile([128, 1], F32, name=f"amn{comp}{rb}")
                    nc.vector.tensor_mul(out=amn[:], in0=am[:], in1=inv[:])
                    nc.vector.tensor_scalar_max(out=amn[:], in0=amn[:],
                                                scalar1=1e-5)
                    rsn = stats.tile([128, 1], F32, name=f"rsn{comp}{rb}")
                    nc.vector.reciprocal(rsn[:], amn[:])
                    nc.vector.tensor_scalar_mul(out=rsn[:], in0=rsn[:],
                                                scalar1=127.0)
                    cq = stats.tile([128, 1], F32, name=f"cq{comp}{rb}")
                    nc.vector.tensor_mul(out=cq[:], in0=inv[:], in1=rsn[:])
                    sc[f"cq{comp}"] = cq
                    sc[f"amn{comp}"] = amn
                for nm, const, amn in (
                    ("d1", rm_d, sc["amnr"]), ("d2", im_d, sc["amni"]),
                    ("d3", im_d, sc["amnr"]), ("d4", rm_d, sc["amni"]),
                ):
                    t = stats.tile([128, 1], F32, name=f"ds{nm}{rb}")
                    nc.vector.tensor_scalar_mul(out=t[:], in0=amn[:],
                                                scalar1=const / 127.0)
                    sc[nm] = t
                dsc[rb] = sc

                for comp, t_d in (("r", tr_d), ("i", ti_d)):
                    nqt_t = nqt_pool.tile([128, IO, 128], F16,
                                          name=f"nqt{comp}{rb}")
                    for ci, (io0, nio) in enumerate(QCH):
                        cw = nio * 128
                        c0 = io0 * 128
                        tin = p2s.tile([128, 7 * 128], F16, tag="tin",
                                       name=f"tin{comp}{rb}_{ci}")
                        nc.sync.dma_start(
                            tin[:, :cw], t_d.ap()[rr:rr + 128, c0:c0 + cw])
                        s1 = p2s.tile([128, 7 * 128], F32, tag="s1q",
                                      name=f"s1q{comp}{rb}_{ci}")
                        nc.scalar.activation(s1[:, :cw], tin[:, :cw],
                                             Act.Copy, bias=MAGIC,
                                             scale=sc[f"cq{comp}"][:])
                        nq = p2s.tile([128, 7 * 128], F16, tag="nq",
                                      name=f"nq{comp}{rb}_{ci}")
                        nc.vector.tensor_scalar(
                            out=nq[:, :cw], in0=s1[:, :cw], scalar1=MAGIC,
                            scalar2=None, op0=Alu.subtract, op1=Alu.bypass)
                        nc.scalar.dma_start_transpose(
                            nqt_t[:, io0:io0 + nio, :], nq[:, :cw])
                    nqt[comp][rb] = nqt_t
                if ri < 2:
                    dtw[ri] = load_dt(ri)

            for ht, (h0, hw) in enumerate(HID_TILES):
                dtr, dti = dtw[ht] if ht in dtw else load_dt(ht)
                for rb in rbs:
                    rr = rb * 128
                    pd = {k: psd.tile([128, HT_W], F32, tag=f"pd{k}",
                                      name=f"pd{k}_{half}_{ht}_{rb}")
                          for k in ("1", "2", "3", "4")}
                    for io in range(IO):
                        st = io == 0
                        sp = io == IO - 1
                        lr = nqt["r"][rb][:, io, :]
                        li = nqt["i"][rb][:, io, :]
                        nc.tensor.matmul(pd["1"][:], lr, dtr[:, io, :],
                                         start=st, stop=sp)
                        nc.tensor.matmul(pd["3"][:], lr, dti[:, io, :],
                                         start=st, stop=sp)
                        nc.tensor.matmul(pd["2"][:], li, dti[:, io, :],
                                         start=st, stop=sp)
                        nc.tensor.matmul(pd["4"][:], li, dtr[:, io, :],
                                         start=st, stop=sp)
                    sc = dsc[rb]
                    ob = ospool.tile([128, HT_W], F32, tag="ob",
                                     name=f"ob{half}_{ht}_{rb}")
                    nc.scalar.activation(ob[:], pd["2"][:], Act.Copy,
                                         scale=sc["d2"][:])
                    o_r = opool.tile([128, HT_W], F32, tag="o_r",
                                     name=f"o_r{half}_{ht}_{rb}")
                    nc.vector.scalar_tensor_tensor(
                        out=o_r[:], in0=pd["1"][:], scalar=sc["d1"][:],
                        in1=ob[:], op0=Alu.mult, op1=Alu.add)
                    nc.gpsimd.dma_start(
                        or_d.ap()[rr:rr + 128, h0:h0 + hw], o_r[:])
                    ob2 = ospool.tile([128, HT_W], F32, tag="ob2",
                                      name=f"ob2{half}_{ht}_{rb}")
                    nc.scalar.activation(ob2[:], pd["4"][:], Act.Copy,
                                         scale=sc["d4"][:])
                    o_i = opool.tile([128, HT_W], F32, tag="o_i",
                                     name=f"o_i{half}_{ht}_{rb}")
                    nc.vector.scalar_tensor_tensor(
                        out=o_i[:], in0=pd["3"][:], scalar=sc["d3"][:],
                        in1=ob2[:], op0=Alu.mult, op1=Alu.subtract)
                    nc.gpsimd.dma_start(
                        oi_d.ap()[rr:rr + 128, h0:h0 + hw], o_i[:])

    nc.compile()
    return nc


_CACHE = {}


def _get_program(key):
    if key not in _CACHE:
        _CACHE[key] = _build_program(*key)
    return _CACHE[key]


_WCACHE = {}


def kernel(x_real, x_imag, gate_wr, gate_wi, up_wr, up_wi,
           down_wr, down_wi, ln_wr, ln_wi, **run_kwargs):
    tgr, tgi, rm_g, im_g = _weight_prep(np.asarray(gate_wr), np.asarray(gate_wi))
    tur, tui, rm_u, im_u = _weight_prep(np.asarray(up_wr), np.asarray(up_wi))
    tdr, tdi, rm_d, im_d = _weight_prep(np.asarray(down_wr), np.asarray(down_wi))
    key = (rm_g, im_g, rm_u, im_u, rm_d, im_d)

    nc = _get_program(key)

    if key not in _WCACHE:
        shared = {}
        for k, tiles in (
            ("gr", _pack_p1(tgr)), ("gi", _pack_p1(tgi)),
            ("gs", _pack_p1(tgr * rm_g + tgi * im_g)),
            ("ur", _pack_p1(tur)), ("ui", _pack_p1(tui)),
            ("us", _pack_p1(tur * rm_u + tui * im_u)),
        ):
            for it, arr in enumerate(tiles):
                shared[f"w{k}_{it}"] = arr
        for k, tiles in (("dr", _pack_dn(tdr)), ("di", _pack_dn(tdi))):
            for ht, arr in enumerate(tiles):
                shared[f"w{k}_{ht}"] = arr
        shared["ln_r"] = np.asarray(ln_wr, np.float32).reshape(1, IM)
        shared["ln_i"] = np.asarray(ln_wi, np.float32).reshape(1, IM)
        _WCACHE.clear()
        _WCACHE[key] = shared
    shared = _WCACHE[key]

    xr = np.ascontiguousarray(np.asarray(x_real, np.float32).reshape(ROWS, HIDDEN))
    xi = np.ascontiguousarray(np.asarray(x_imag, np.float32).reshape(ROWS, HIDDEN))

    in_maps = []
    for c in range(NCORES):
        sl = slice(c * RPC, (c + 1) * RPC)
        in_maps.append({"xr": np.ascontiguousarray(xr[sl]),
                        "xi": np.ascontiguousarray(xi[sl]), **shared})

    res = run_bass_kernel_spmd(nc, in_maps, core_ids=list(range(NCORES)),
                               **run_kwargs)

    out_r = np.concatenate([res.results[c]["o_r"] for c in range(NCORES)],
                           axis=0).reshape(B, S, HIDDEN)
    out_i = np.concatenate([res.results[c]["o_i"] for c in range(NCORES)],
                           axis=0).reshape(B, S, HIDDEN)
    kernel.last_results = res
    return out_r, out_i
